# revision 9
# baseline (speedup 1.0000x reference)
"""CrissCrossAttention (channel-attention variant) Trainium2 Bass kernel.

Reference computation (per batch b, NUM_HEADS=2, C=256, H=W=128, n=H*W=16384):
    q = Wq x + bq ; k = Wk x + bk ; v = Wv x + bv        (1x1 convs, x: [C, n])
    A_h = q_h k_h^T          [d, d] per head (d=128), contraction over n
    attn = softmax(A, -1)
    out_h = attn_h v_h       [d, n]
    y = gamma * out + x

Algebraic restructuring used here (exactly equivalent):
    With Ghat = [[X X^T, X 1], [1^T X^T, n]]  ([C+1, C+1], symmetric) and the
    bias-augmented weights What_h = [W_h | b_h]  ([d, C+1]):
        A_h  = Whatq_h  Ghat  Whatk_h^T
        out  = M x + c 1^T,  M_h = attn_h Wv_h,  c_h = attn_h bv_h
        y    = x + (gamma M) x + (gamma c) 1^T
    So the big-n work is only: (1) the Gram matrix G = X X^T (+ row sums via a
    ones column), and (2) one final [256,256] @ [256,n] projection.

Sharding: data-parallel over batch B=8 across the 8 NeuronCores (1 batch per
core), weights replicated, no cross-core communication.

Per-core phases:
  P1: stream x [256, 16384] into SBUF; PE-transpose 128-column tiles and
      accumulate Ghat in PSUM (fp32r matmuls, N=258 -> full PE rate).
  P2: tiny [<=257 x <=257] algebra: A_h, softmax, M_h, c_h -> WfT = gamma*M^T.
  P3: y = x + WfT^T x + c' 1^T, streamed back out (fp32r matmuls, N=512).

fp32r notes (walrus-enforced): every matmul input must be produced by an
instruction with fp32r output dtype (DVE copy f32->f32r rounds; DMA into an
f32r-declared DRAM tensor also qualifies), and fp32r matmul free size must be
even. x lives in SBUF as f32r (raw f32 bits from DMA); non-matmul consumers
read it via .bitcast(f32) so the residual +x stays full precision.
"""

import sys

if "/opt/trn_rl_repo" not in sys.path:
    sys.path.insert(0, "/opt/trn_rl_repo")

import numpy as np

B, C, H, W = 8, 256, 128, 128
NPIX = H * W            # 16384
P = 128                 # partitions
NT = NPIX // P          # 128 transpose tiles
LOAD_CHUNK = 1024       # x DMA chunk (free dim)
OUT_CHUNK = 512         # phase-3 chunk (free dim, one PSUM bank of fp32)
N_CORES = 8

_cache = {}


def _build_program(gamma_f: float):
    import concourse.bass as bass
    import concourse.mybir as mybir
    import concourse.tile as tile
    from concourse import bacc
    from concourse.masks import make_identity

    f32 = mybir.dt.float32
    f32r = mybir.dt.float32r
    AF = mybir.ActivationFunctionType
    AX = mybir.AxisListType
    ALU = mybir.AluOpType

    nc = bacc.Bacc(
        "TRN2",
        target_bir_lowering=False,
        debug=False,
        enable_asserts=False,
    )

    x_d = nc.dram_tensor("x", (C, NPIX), f32r, kind="ExternalInput").ap()
    wq_d = nc.dram_tensor("Wq", (C, C), f32, kind="ExternalInput").ap()
    bq_d = nc.dram_tensor("bq", (C,), f32, kind="ExternalInput").ap()
    wk_d = nc.dram_tensor("Wk", (C, C), f32, kind="ExternalInput").ap()
    bk_d = nc.dram_tensor("bk", (C,), f32, kind="ExternalInput").ap()
    wv_d = nc.dram_tensor("Wv", (C, C), f32, kind="ExternalInput").ap()
    bv_d = nc.dram_tensor("bv", (C,), f32, kind="ExternalInput").ap()
    y_d = nc.dram_tensor("y", (C, NPIX), f32, kind="ExternalOutput").ap()

    with tile.TileContext(nc) as tc:
        with tc.tile_pool(name="const", bufs=1) as const:
            ident = const.tile([P, P], f32, tag="ident")
            make_identity(nc, ident)
            identr = const.tile([P, P], f32r, tag="identr")
            nc.vector.tensor_copy(identr[:], ident[:])
            # [ones | zeros] pad columns for the Gram rhs
            onespad = const.tile([P, 2], f32, tag="onespad")
            nc.gpsimd.memset(onespad[:, 0:1], 1.0)
            nc.gpsimd.memset(onespad[:, 1:2], 0.0)

            # Replicated weights FIRST (small; must not queue behind the 16MiB
            # x stream — the W transposes are the first ops on the in-order PE
            # stream). WqT/WkT hold W^T ([c, o] layout); Wv natural.
            WqT = const.tile([P, 2, C], f32, tag="WqT")
            WkT = const.tile([P, 2, C], f32, tag="WkT")
            Wv_sb = const.tile([P, 2, C], f32, tag="Wv_sb")
            nc.sync.dma_start(Wv_sb[:], wv_d.rearrange("(t p) c -> p t c", p=P))
            bq_row = const.tile([1, C], f32, tag="bq_row")
            bk_row = const.tile([1, C], f32, tag="bk_row")
            nc.sync.dma_start(bq_row[:], bq_d.rearrange("(o c) -> o c", o=1))
            nc.sync.dma_start(bk_row[:], bk_d.rearrange("(o c) -> o c", o=1))
            bv_col = const.tile([P, 2], f32, tag="bv_col")
            nc.sync.dma_start(bv_col[:], bv_d.rearrange("(t p) -> p t", p=P))

            # x resident in SBUF for the whole kernel: [p, c_tile, n] (f32r,
            # raw f32 bits; matmuls read natively, others via bitcast)
            x_sb = const.tile([P, 2, NPIX], f32r, tag="x_sb")
            for j in range(NPIX // LOAD_CHUNK):
                sl = slice(j * LOAD_CHUNK, (j + 1) * LOAD_CHUNK)
                for ch in range(2):
                    nc.sync.dma_start(
                        x_sb[:, ch, sl], x_d[ch * P:(ch + 1) * P, sl]
                    )

            # Ghat = [[G, s], [s^T, n]]; rows 0:128 / 128:256 / 256.
            Ghat0 = const.tile([P, C + 1], f32, tag="Ghat0")
            Ghat1 = const.tile([P, C + 1], f32, tag="Ghat1")
            Ghat2 = const.tile([1, C + 1], f32, tag="Ghat2")

            # Final projection (gamma * M)^T as [c_inner, c_tile, o] (f32r,
            # written by DVE scalar-mul which rounds) and the bias column.
            WfT = const.tile([P, 2, C], f32r, tag="WfT")
            cp_col = const.tile([P, 2], f32, tag="cp_col")

            # ---------------- Phase 1: W transposes + Gram matrix ----------
            with tc.tile_pool(name="ph1sb", bufs=2) as wtmp, \
                 tc.tile_pool(name="xtp", bufs=3) as xtp, \
                 tc.tile_pool(name="ps1", bufs=1, space="PSUM") as ps1:

                # W^T via PE transposes (one-time, small, fp32)
                for w_dram, wt_sb in ((wq_d, WqT), (wk_d, WkT)):
                    wnat = wtmp.tile([P, 2, C], f32, tag="wnat", bufs=2)
                    nc.sync.dma_start(
                        wnat[:], w_dram.rearrange("(t p) c -> p t c", p=P)
                    )
                    for ct in range(2):
                        for ot in range(2):
                            tp = ps1.tile([P, P], f32, tag="tp", bufs=4)
                            nc.tensor.transpose(
                                tp[:], wnat[:, ot, ct * P:(ct + 1) * P], ident[:]
                            )
                            nc.vector.tensor_copy(
                                wt_sb[:, ct, ot * P:(ot + 1) * P], tp[:]
                            )

                g_ps0 = ps1.tile([P, C + 2], f32, tag="g0", bufs=1)
                g_ps1 = ps1.tile([P, C + 2], f32, tag="g1", bufs=1)

                for it in range(NT):
                    sl = slice(it * P, (it + 1) * P)
                    xt = xtp.tile([P, C + 2], f32r, tag="xt", bufs=4)
                    nc.vector.tensor_copy(xt[:, C:C + 2], onespad[:])
                    for ch in range(2):
                        tpr = ps1.tile([P, P], f32r, tag="tp", bufs=4)
                        nc.tensor.transpose(tpr[:], x_sb[:, ch, sl], identr[:])
                        # split PSUM->SBUF rounding copies across DVE and ACT
                        # so neither engine stalls the PE stream
                        if ch == 0:
                            nc.vector.tensor_copy(xt[:, 0:P], tpr[:])
                        else:
                            nc.scalar.activation(
                                xt[:, P:2 * P], tpr[:], AF.Copy,
                                bias=0.0, scale=1.0,
                            )
                    nc.tensor.matmul(
                        g_ps0[:], lhsT=xt[:, 0:P], rhs=xt[:],
                        start=(it == 0), stop=(it == NT - 1),
                    )
                    nc.tensor.matmul(
                        g_ps1[:], lhsT=xt[:, P:2 * P], rhs=xt[:],
                        start=(it == 0), stop=(it == NT - 1),
                    )

                nc.vector.tensor_copy(Ghat0[:], g_ps0[:, 0:C + 1])
                nc.vector.tensor_copy(Ghat1[:], g_ps1[:, 0:C + 1])

            # ---------------- Phase 2: heads, softmax, WfT -----------------
            with tc.tile_pool(name="midsb", bufs=1) as msb, \
                 tc.tile_pool(name="ps2", bufs=1, space="PSUM") as ps2:

                # Bottom Ghat row [s^T, n] from the s columns.
                for ch, gh in ((0, Ghat0), (1, Ghat1)):
                    tsp = ps2.tile([1, P], f32, tag="tsp", bufs=1)
                    nc.tensor.transpose(tsp[:], gh[:, C:C + 1], ident[:])
                    nc.vector.tensor_copy(Ghat2[0:1, ch * P:(ch + 1) * P], tsp[:])
                nc.gpsimd.memset(Ghat2[0:1, C:C + 1], float(NPIX))

                ghat_k = (Ghat0, Ghat1, Ghat2)
                for h in range(2):
                    osl = slice(h * P, (h + 1) * P)
                    # Phat = Ghat @ WhatkT[:, osl]  -> [257, 128]
                    P_sb = msb.tile([P, 2, P], f32, tag=f"P_sb{h}")
                    P_row = msb.tile([1, P], f32, tag=f"P_row{h}")
                    wkt_k = (WkT[:, 0, osl], WkT[:, 1, osl], bk_row[0:1, osl])
                    for m in range(3):
                        mp = P if m < 2 else 1
                        msl = slice(m * P, m * P + mp) if m < 2 else slice(C, C + 1)
                        pps = ps2.tile([mp, P], f32, tag="pps", bufs=2)
                        for k in range(3):
                            gk = ghat_k[k]
                            nc.tensor.matmul(
                                pps[:], lhsT=gk[:, msl], rhs=wkt_k[k],
                                start=(k == 0), stop=(k == 2),
                            )
                        if m < 2:
                            nc.vector.tensor_copy(P_sb[:, m, :], pps[:])
                        else:
                            nc.vector.tensor_copy(P_row[:], pps[:])

                    # A = WhatqT[:, osl].T @ Phat -> [128, 128]
                    aps = ps2.tile([P, P], f32, tag="aps", bufs=1)
                    wqt_k = (WqT[:, 0, osl], WqT[:, 1, osl], bq_row[0:1, osl])
                    p_k = (P_sb[:, 0, :], P_sb[:, 1, :], P_row[0:1, :])
                    for k in range(3):
                        nc.tensor.matmul(
                            aps[:], lhsT=wqt_k[k], rhs=p_k[k],
                            start=(k == 0), stop=(k == 2),
                        )

                    # Softmax along free dim.
                    negmax = msb.tile([P, 1], f32, tag="negmax")
                    nc.vector.tensor_reduce(
                        negmax[:], aps[:], axis=AX.X, op=ALU.max, negate=True
                    )
                    exp_sb = msb.tile([P, P], f32, tag="exp_sb")
                    sumexp = msb.tile([P, 1], f32, tag="sumexp")
                    nc.scalar.activation(
                        exp_sb[:], aps[:], AF.Exp,
                        bias=negmax[:], scale=1.0, accum_out=sumexp[:],
                    )
                    rinv = msb.tile([P, 1], f32, tag="rinv")
                    nc.vector.reciprocal(rinv[:], sumexp[:])
                    attn = msb.tile([P, P], f32, tag="attn")
                    nc.vector.tensor_scalar_mul(attn[:], exp_sb[:], rinv[:])

                    tat = ps2.tile([P, P], f32, tag="tat", bufs=1)
                    nc.tensor.transpose(tat[:], attn[:], ident[:])
                    attnT = msb.tile([P, P], f32, tag="attnT")
                    nc.vector.tensor_copy(attnT[:], tat[:])

                    # M^T blocks: Wv_h[:, ct*P:...].T @ attnT -> [c, d]
                    for ct in range(2):
                        mps = ps2.tile([P, P], f32, tag="mps", bufs=2)
                        nc.tensor.matmul(
                            mps[:], lhsT=Wv_sb[:, h, ct * P:(ct + 1) * P],
                            rhs=attnT[:], start=True, stop=True,
                        )
                        nc.vector.tensor_scalar_mul(
                            WfT[:, ct, osl], mps[:], gamma_f
                        )
                    # c_h = attn_h bv_h: rhs = [bv_0 | bv_1], keep column h
                    cps = ps2.tile([P, 2], f32, tag="cps", bufs=1)
                    nc.tensor.matmul(
                        cps[:], lhsT=attnT[:], rhs=bv_col[:],
                        start=True, stop=True,
                    )
                    nc.vector.tensor_scalar_mul(
                        cp_col[:, h:h + 1], cps[:, h:h + 1], gamma_f
                    )

            # ---------------- Phase 3: y = x + WfT^T x + c' ----------------
            with tc.tile_pool(name="outsb", bufs=1) as osb, \
                 tc.tile_pool(name="ps3", bufs=1, space="PSUM") as ps3:
                for j in range(NPIX // OUT_CHUNK):
                    nsl = slice(j * OUT_CHUNK, (j + 1) * OUT_CHUNK)
                    for oh in range(2):
                        yps = ps3.tile([P, OUT_CHUNK], f32, tag=f"y{oh}", bufs=2)
                        for ch in range(2):
                            nc.tensor.matmul(
                                yps[:],
                                lhsT=WfT[:, ch, oh * P:(oh + 1) * P],
                                rhs=x_sb[:, ch, nsl],
                                start=(ch == 0), stop=(ch == 1),
                            )
                        t_sb = osb.tile([P, OUT_CHUNK], f32, tag=f"t{oh}", bufs=3)
                        nc.scalar.activation(
                            t_sb[:], yps[:], AF.Identity,
                            bias=cp_col[:, oh:oh + 1], scale=1.0,
                        )
                        y_sb = osb.tile([P, OUT_CHUNK], f32, tag=f"yo{oh}", bufs=3)
                        nc.vector.tensor_add(
                            out=y_sb[:], in0=t_sb[:],
                            in1=x_sb.bitcast(f32)[:, oh, nsl],
                        )
                        nc.sync.dma_start(y_d[oh * P:(oh + 1) * P, nsl], y_sb[:])

    nc.compile()
    return nc


def _get_program(gamma_f: float):
    key = ("v3", gamma_f)
    if key not in _cache:
        _cache[key] = _build_program(gamma_f)
    return _cache[key]


def _run(inputs: dict, trace: bool = False):
    from concourse import bass_utils

    x = np.ascontiguousarray(np.asarray(inputs["x"], dtype=np.float32))
    gamma_f = float(np.asarray(inputs["gamma"]).reshape(-1)[0])
    nc = _get_program(gamma_f)

    weights = {
        name: np.ascontiguousarray(np.asarray(inputs[name], dtype=np.float32))
        for name in ("Wq", "bq", "Wk", "bk", "Wv", "bv")
    }
    in_maps = []
    for b in range(N_CORES):
        m = dict(weights)
        m["x"] = x[b].reshape(C, NPIX)
        in_maps.append(m)

    res = bass_utils.run_bass_kernel_spmd(
        nc, in_maps, core_ids=list(range(N_CORES)), trace=trace
    )
    out = np.stack(
        [res.results[b]["y"].reshape(C, H, W) for b in range(N_CORES)]
    ).astype(np.float32)
    return out, res


def kernel(**inputs) -> np.ndarray:
    out, _ = _run(inputs, trace=False)
    return out
